# revision 41
# baseline (speedup 1.0000x reference)
# Fused dynamic-conv (CondInst-style) + dice loss kernel for 8x TRN2 NeuronCores.
#
# Reference computation (per batch image b, object o):
#   weight[b,o,:] = conv_weight[b, :, ind[b,o]]           (gather, 593 params)
#   feat = concat(seg_feat[b], x_rel(o), y_rel(o))        ([18, 128*128])
#   h1 = relu(w1 @ feat + b1); h2 = relu(w2 @ h1 + b2)    (16-ch dynamic 1x1 convs)
#   out = sigmoid(w3 . h2 + b3)                           ([128*128])
#   dice over masked objects -> scalar loss
#
# Strategy (v2):
#  * Host gathers the 593 params per object, packs active objects into groups
#    of 8 per image (G groups total).  x_rel/y_rel fold into b1_eff.
#  * Work unit = "wave" = 4 pixel-tiles of 512 px of one group.  Each group
#    has 32 tiles = 8 waves; wave j of every group goes to core j -> every
#    core gets exactly G waves (perfect balance).
#  * gemm1 runs as 16 concurrent 32x32 PE sub-array tiles: feat for the 4
#    pixel-tiles is STACKED in partition bands 32r (18 rows each, no
#    replication); MM (r,c) = tile r x object-pair c -> PSUM bank r holds
#    canonical h1 (obj o at partitions 16o) of tile r.
#  * gemm2 is block-diagonal per pair on the 4 diagonal sub-arrays (x,x),
#    reading canonical h1, writing canonical h2 -> bank r = tile r.
#  * evac1/evac2 are single [128,1024] relu+bias ops (fp32 PSUM -> fp16 SBUF),
#    alternating between DVE (tensor_scalar add+max) and ACT (activation Relu)
#    per half/wave so both engines stay saturated - they are the bottleneck.
#  * gemm3 accumulates 4 MMs per wave (one per tile, w3 block placed at
#    output columns 8r..8r+8) into a DENSE [32,512] pred quadrant: rows =
#    4 tiles x 8 objects, cols = 512 px of that row's tile.  4 waves fill a
#    [128,512] PSUM bank -> ONE sigmoid + 2 accumulating dice ops per 4 waves
#    (4x less pointwise work than per-tile layouts).
#  * Dice partials: sigmoid (ACT, bias b3, -50 on filler rows) then
#    scalar_tensor_tensor pred*tgt and pred*pred with accum_out (DVE, fp16
#    2x mode).  Host does the final tiny reduction + sum(tgt^2).
import math
import numpy as np
from contextlib import ExitStack

import concourse.bass as bass
import concourse.tile as tile
from concourse import mybir, bacc
from concourse.bass_utils import run_bass_kernel_spmd

C = 16
WT = 593
B, O, H, W = 4, 32, 128, 128
HW = H * W
N_CORES = 8
GRP = 8            # objects per group (4 pairs)
PXT = 512          # pixels per tile
TPW = 4            # tiles per wave
WPG = HW // (PXT * TPW)  # waves per group = 8 (== N_CORES)

F32 = mybir.dt.float32
F16 = mybir.dt.float16
ACTF = mybir.ActivationFunctionType
ALU = mybir.AluOpType

# per-group weight columns in the resident wtile (fp16):
#   0:128   w1pack: [32r+k, 32c+j] = w1^T of pair c (k<18), replicated per band r
#   128:160 w2pack: [32x+k, j]     = blockdiag(w2[2x]^T, w2[2x+1]^T)
#   160:288 w3pack: cols 32r+m     = lhsT3 variant r ([16o+ch, 8r+o] = w3[o,ch])
WCOL = 288


def host_pack(seg_feat, conv_weight, mask, ind, target):
    cw = conv_weight.reshape(B, WT, HW)
    weight = np.take_along_axis(cw, ind[:, None, :].astype(np.int64), axis=2)
    weight = np.ascontiguousarray(weight.transpose(0, 2, 1))  # [B, O, WT]
    s0 = (C + 2) * C
    w1 = weight[..., :s0].reshape(B, O, C, C + 2)
    b1 = weight[..., s0:s0 + C]
    w2 = weight[..., s0 + C:s0 + C + C * C].reshape(B, O, C, C)
    b2 = weight[..., s0 + C + C * C:s0 + 2 * C + C * C]
    w3 = weight[..., s0 + 2 * C + C * C:s0 + 3 * C + C * C]
    b3 = weight[..., -1]
    xo = (ind % W).astype(np.float32)
    yo = (ind // W).astype(np.float32)

    groups = []  # (b, [obj ids padded with -1])
    for b in range(B):
        objs = [o for o in range(O) if mask[b, o] == 1]
        for g0 in range(0, len(objs), GRP):
            grp = objs[g0:g0 + GRP]
            groups.append((b, grp + [-1] * (GRP - len(grp))))
    if not groups:
        groups.append((0, [-1] * GRP))
    G = len(groups)
    NB = (G + 3) // 4
    KLAST = G - 4 * (NB - 1)  # quadrants in last batch

    px = np.arange(HW, dtype=np.float32)
    xg = (px % W) / 128.0
    yg = np.floor(px / W) / 128.0
    tgt_flat = target.reshape(B, O, HW)

    # group-level packs (core independent)
    wall = np.zeros((G, 128, WCOL), np.float16)
    ball = np.zeros((G, 128, 2), np.float32)
    for g, (b, grp) in enumerate(groups):
        for oo, o in enumerate(grp):
            if o < 0:
                continue
            pr, sub = oo // 2, oo % 2  # pair index, slot in pair
            # w1pack: bands r, block col 32*pr, cols 16*sub..
            blk = w1[b, o].T.astype(np.float16)  # [18, 16]
            for r in range(4):
                wall[g, 32 * r:32 * r + 18, 32 * pr + 16 * sub:
                     32 * pr + 16 * sub + 16] = blk
            # w2pack at rows 32*pr (+16*sub), cols 128 + 16*sub
            wall[g, 32 * pr + 16 * sub:32 * pr + 16 * sub + 16,
                 128 + 16 * sub:128 + 16 * sub + 16] = \
                w2[b, o].T.astype(np.float16)
            b1e = (b1[b, o] - w1[b, o, :, 16] * (xo[b, o] / 128.0)
                   - w1[b, o, :, 17] * (yo[b, o] / 128.0))
            ball[g, 16 * oo:16 * oo + 16, 0] = b1e
            ball[g, 16 * oo:16 * oo + 16, 1] = b2[b, o]
    # w3pack: variant r lives at cols 160+32r..160+32r+32; within it the
    # nonzero column for (tile-slot r, object oo) is m = 8r+oo.
    for g, (b, grp) in enumerate(groups):
        for oo, o in enumerate(grp):
            if o < 0:
                continue
            for r in range(4):
                wall[g, 16 * oo:16 * oo + 16,
                     160 + 32 * r + 8 * r + oo] = w3[b, o].astype(np.float16)

    # transpose-to-[128, G*...] layouts so weights/biases load in ONE DMA
    wall_t = np.ascontiguousarray(wall.transpose(1, 0, 2)).reshape(128, G * WCOL)
    ball_t = np.ascontiguousarray(ball.transpose(1, 0, 2)).reshape(128, G * 2)

    in_maps = []
    meta = {"groups": groups, "G": G, "NB": NB, "KLAST": KLAST}
    for ci in range(N_CORES):
        feat_pack = np.zeros((G, 128, PXT), np.float16)
        tgt_pack = np.zeros((NB, 128, PXT), np.float16)
        b3_pack = np.full((128, NB), -50.0, np.float32)
        for g, (b, grp) in enumerate(groups):
            for r in range(TPW):
                t = TPW * ci + r
                sl = slice(t * PXT, (t + 1) * PXT)
                feat_pack[g, 32 * r:32 * r + 16] = seg_feat[b].reshape(C, HW)[:, sl]
                feat_pack[g, 32 * r + 16] = xg[sl]
                feat_pack[g, 32 * r + 17] = yg[sl]
            nb, q = g // 4, g % 4
            for r in range(TPW):
                t = TPW * ci + r
                for oo, o in enumerate(grp):
                    if o < 0:
                        continue
                    row = 32 * q + 8 * r + oo
                    tgt_pack[nb, row] = tgt_flat[b, o, t * PXT:(t + 1) * PXT
                                                 ].astype(np.float16)
                    b3_pack[row, nb] = b3[b, o]
        in_maps.append({"feat": feat_pack, "wall": wall_t, "ball": ball_t,
                        "tgt": tgt_pack, "b3": b3_pack})
    return in_maps, meta


_PROGRAM_CACHE = {}


def build_program(G, NB, KLAST):
    key = (G, NB, KLAST)
    if key in _PROGRAM_CACHE:
        return _PROGRAM_CACHE[key]
    nc = bacc.Bacc("TRN2", target_bir_lowering=False, debug=False,
                   enable_asserts=False, num_devices=N_CORES)
    feat_t = nc.dram_tensor("feat", (G, 128, PXT), F16, kind="ExternalInput")
    wall_t = nc.dram_tensor("wall", (128, G * WCOL), F16, kind="ExternalInput")
    ball_t = nc.dram_tensor("ball", (128, G * 2), F32, kind="ExternalInput")
    tgt_t = nc.dram_tensor("tgt", (NB, 128, PXT), F16, kind="ExternalInput")
    b3_t = nc.dram_tensor("b3", (128, NB), F32, kind="ExternalInput")
    acc_t = nc.dram_tensor("acc", (128, 2 * NB), F32, kind="ExternalOutput")

    with tile.TileContext(nc) as tc, ExitStack() as ctx:
        wpool = ctx.enter_context(tc.tile_pool(name="wpool", bufs=1))
        fpool = ctx.enter_context(tc.tile_pool(name="fpool", bufs=4))
        h1pool = ctx.enter_context(tc.tile_pool(name="h1pool", bufs=2))
        h2pool = ctx.enter_context(tc.tile_pool(name="h2pool", bufs=5))
        spool = ctx.enter_context(tc.tile_pool(name="spool", bufs=2))
        apool = ctx.enter_context(tc.tile_pool(name="apool", bufs=1))
        ps1 = ctx.enter_context(tc.tile_pool(name="ps1", bufs=1, space="PSUM"))
        ps2 = ctx.enter_context(tc.tile_pool(name="ps2", bufs=1, space="PSUM"))

        acc_sb = apool.tile([128, 2 * NB], F32)
        inter_acc = acc_sb[:, 0:NB]
        psq_acc = acc_sb[:, NB:2 * NB]

        # wave-0 inputs first on the sync queue; per-group weight slices as
        # individual small DMAs so wave g's weights land long before wave g
        ft_tiles = {}
        ft_tiles[0] = fpool.tile([128, PXT], F16, tag="f", name="ft0")
        nc.sync.dma_start(out=ft_tiles[0], in_=feat_t.ap()[0])
        wts = []
        for g in range(G):
            wtg = wpool.tile([128, WCOL], F16, tag=f"w{g}", name="wtg")
            wts.append(wtg)
        bt = wpool.tile([128, 2 * G], F32)
        b3t = wpool.tile([128, NB], F32)
        nc.sync.dma_start(out=wts[0], in_=wall_t.ap()[:, 0:WCOL])
        nc.gpsimd.dma_start(out=bt, in_=ball_t.ap())
        nc.gpsimd.dma_start(out=b3t, in_=b3_t.ap())
        if G > 1:
            ft_tiles[1] = fpool.tile([128, PXT], F16, tag="f", name="ft1")
            nc.gpsimd.dma_start(out=ft_tiles[1], in_=feat_t.ap()[1])
        for g in range(1, G):
            nc.sync.dma_start(out=wts[g],
                              in_=wall_t.ap()[:, WCOL * g:WCOL * (g + 1)])

        def wslice(g, lo, hi):
            return wts[g][:, lo:hi]

        # dice-batch inputs prefetched on the gpsimd queue
        tg_tiles = []
        for nb in range(NB):
            tg = spool.tile([128, PXT], F16, tag="t", bufs=NB, name="tg")
            nc.gpsimd.dma_start(out=tg, in_=tgt_t.ap()[nb])
            tg_tiles.append(tg)

        # sigmoid table set (covers relu/square) loads on ACT during the
        # initial DMA wait
        scr = apool.tile([128, 512], F16)
        nc.vector.memset(scr[0:1, 0:1], 0.125)
        scr1 = apool.tile([128, 1], F32)
        nc.scalar.activation(scr1[0:1, :], scr[0:1, 0:1], ACTF.Sigmoid,
                             bias=0.0, scale=1.0)

        def evac_half(engine, dst, src, bias_ap):
            if engine == 0:
                nc.vector.tensor_scalar(out=dst, in0=src, scalar1=bias_ap,
                                        scalar2=0.0, op0=ALU.add, op1=ALU.max)
            else:
                nc.scalar.activation(dst, src, ACTF.Relu, bias=bias_ap,
                                     scale=1.0)

        h1_of = {}
        h2_tiles = [None] * 4
        p1_of = {}
        pending_dice = []

        # Software-pipelined: iteration w runs gemm1/evac1 of wave w and
        # gemm2/evac2 (+dice batch) of wave w-1, so each PE burst
        # (g1a,g1b,g2a,g2b) hides under the previous engine phase.
        for w in range(G + 1):
            if w + 2 < G:
                ftn = fpool.tile([128, PXT], F16, tag="f", name="ftn")
                if w % 2 == 0:
                    nc.sync.dma_start(out=ftn, in_=feat_t.ap()[w + 2])
                else:
                    nc.gpsimd.dma_start(out=ftn, in_=feat_t.ap()[w + 2])
                ft_tiles[w + 2] = ftn

            if w < G:
                ft = ft_tiles.pop(w)
                p1a = ps1.tile([128, 1024], F32, tag="g1a", name="p1a")
                p1b = ps1.tile([128, 1024], F32, tag="g1b", name="p1b")
                for r in range(TPW):
                    p1s = p1a if r < 2 else p1b
                    for c2 in range(4):
                        nc.tensor.matmul(
                            p1s[32 * c2:32 * c2 + 32,
                                512 * (r % 2):512 * (r % 2) + 512],
                            wslice(w, 0, 128)[32 * r:32 * r + 18,
                                              32 * c2:32 * c2 + 32],
                            ft[32 * r:32 * r + 18, :],
                            start=True, stop=True,
                            tile_position=(32 * r, 32 * c2))
                p1_of[w] = (p1a, p1b)

            if w > 0:
                v = w - 1
                h1a, h1b = h1_of[v]
                p2a = ps2.tile([128, 1024], F32, tag="g2a", name="p2a")
                p2b = ps2.tile([128, 1024], F32, tag="g2b", name="p2b")
                for r in range(TPW):
                    h1s = h1a if r < 2 else h1b
                    p2s = p2a if r < 2 else p2b
                    cc = 512 * (r % 2)
                    for x in range(4):
                        nc.tensor.matmul(
                            p2s[32 * x:32 * x + 32, cc:cc + 512],
                            wslice(v, 128, 160)[32 * x:32 * x + 32, :],
                            h1s[32 * x:32 * x + 32, cc:cc + 512],
                            start=True, stop=True,
                            tile_position=(32 * x, 32 * x))

            if w < G:
                b1ap = bt[:, 2 * w:2 * w + 1]
                h1a = h1pool.tile([128, 1024], F16, tag="h1a", name="h1a")
                h1b = h1pool.tile([128, 1024], F16, tag="h1b", name="h1b")
                p1a, p1b = p1_of.pop(w)
                drop_dve = (w % 8 == 6)
                evac_half(1 if drop_dve else 0, h1a, p1a, b1ap)
                evac_half(1, h1b, p1b, b1ap)
                h1_of[w] = (h1a, h1b)

            if w > 0:
                v = w - 1
                b2ap = bt[:, 2 * v + 1:2 * v + 2]
                h2a = h2pool.tile([128, 1024], F16, tag="h2a", name="h2a")
                h2b = h2pool.tile([128, 1024], F16, tag="h2b", name="h2b")
                evac_half(0, h2a, p2a, b2ap)
                evac_half(1, h2b, p2b, b2ap)
                h2_tiles[v % 4] = (h2a, h2b, v)

                if v % 4 == 3 or v == G - 1:
                    nb = v // 4
                    k = v % 4 + 1
                    pred = ps2.tile([128, 512], F32, tag="g2a", name="pred")
                    for q in range(k):
                        h2qa, h2qb, gq = h2_tiles[q]
                        for r in range(TPW):
                            h2s = h2qa if r < 2 else h2qb
                            cc = 512 * (r % 2)
                            nc.tensor.matmul(
                                pred[32 * q:32 * q + 32, :],
                                wslice(gq, 160 + 32 * r, 192 + 32 * r),
                                h2s[:, cc:cc + 512],
                                start=(r == 0), stop=(r == TPW - 1),
                                tile_position=(0, 32 * q))
                    predsb = spool.tile([128, PXT], F16, tag="p", name="psb")
                    pp = 32 * k
                    nc.scalar.activation(predsb[0:pp, :], pred[0:pp, :],
                                         ACTF.Sigmoid,
                                         bias=b3t[0:pp, nb:nb + 1], scale=1.0)
                    # dice product ops have no downstream consumer until the
                    # final DMA - defer their emission one iteration so the
                    # next wave's evacs schedule ahead of them (they were
                    # stalling the engine FIFOs waiting on sigmoid).
                    pending_dice.append((nb, pp, predsb))

            if pending_dice and (w > 0 and (w - 1) % 4 == 0 or w == G):
                nb, pp, predsb = pending_dice.pop(0)
                sc1 = spool.tile([128, PXT], F16, tag="s1", name="sc1")
                nc.vector.scalar_tensor_tensor(
                    out=sc1[0:pp, :], in0=predsb[0:pp, :], scalar=0.0,
                    in1=tg_tiles[nb][0:pp, :], op0=ALU.add, op1=ALU.mult,
                    accum_out=inter_acc[0:pp, nb:nb + 1])
                sc2 = spool.tile([128, PXT], F16, tag="s2", name="sc2")
                nc.scalar.activation(sc2[0:pp, :], predsb[0:pp, :],
                                     ACTF.Square,
                                     accum_out=psq_acc[0:pp, nb:nb + 1])

        while pending_dice:
            nb, pp, predsb = pending_dice.pop(0)
            sc1 = spool.tile([128, PXT], F16, tag="s1", name="sc1")
            nc.vector.scalar_tensor_tensor(
                out=sc1[0:pp, :], in0=predsb[0:pp, :], scalar=0.0,
                in1=tg_tiles[nb][0:pp, :], op0=ALU.add, op1=ALU.mult,
                accum_out=inter_acc[0:pp, nb:nb + 1])
            sc2 = spool.tile([128, PXT], F16, tag="s2", name="sc2")
            nc.scalar.activation(sc2[0:pp, :], predsb[0:pp, :],
                                 ACTF.Square,
                                 accum_out=psq_acc[0:pp, nb:nb + 1])

        nc.sync.dma_start(out=acc_t.ap(), in_=acc_sb)

    nc.compile()
    _PROGRAM_CACHE[key] = nc
    return nc


def _run(inputs, trace=False):
    seg_feat = np.asarray(inputs["seg_feat"], np.float32)
    conv_weight = np.asarray(inputs["conv_weight"], np.float32)
    mask = np.asarray(inputs["mask"])
    ind = np.asarray(inputs["ind"])
    target = np.asarray(inputs["target"], np.float32)

    in_maps, meta = host_pack(seg_feat, conv_weight, mask, ind, target)
    G, NB, KLAST = meta["G"], meta["NB"], meta["KLAST"]
    groups = meta["groups"]
    nc = build_program(G, NB, KLAST)
    res = run_bass_kernel_spmd(nc, in_maps, core_ids=list(range(N_CORES)),
                               trace=trace)

    inter = np.zeros(B, np.float64)
    predsq = np.zeros(B, np.float64)
    for ci in range(N_CORES):
        acc = res.results[ci]["acc"]
        for g, (b, grp) in enumerate(groups):
            if all(o < 0 for o in grp):
                continue
            nb, q = g // 4, g % 4
            inter[b] += acc[32 * q:32 * q + 32, nb].sum(dtype=np.float64)
            predsq[b] += acc[32 * q:32 * q + 32, NB + nb].sum(dtype=np.float64)
    tgtsq = ((target.reshape(B, O, HW).astype(np.float64) ** 2)
             * mask[:, :, None]).sum(axis=(1, 2))
    loss = 1.0 - (2.0 * inter + 1.0) / (predsq + tgtsq + 1.0)
    return np.float32(loss.mean()), res


def kernel(**inputs):
    loss, _ = _run(inputs, trace=False)
    return np.array(loss, dtype=np.float32)


# revision 43
# speedup vs baseline: 1.0106x; 1.0106x over previous
# Fused dynamic-conv (CondInst-style) + dice loss kernel for 8x TRN2 NeuronCores.
#
# Reference computation (per batch image b, object o):
#   weight[b,o,:] = conv_weight[b, :, ind[b,o]]           (gather, 593 params)
#   feat = concat(seg_feat[b], x_rel(o), y_rel(o))        ([18, 128*128])
#   h1 = relu(w1 @ feat + b1); h2 = relu(w2 @ h1 + b2)    (16-ch dynamic 1x1 convs)
#   out = sigmoid(w3 . h2 + b3)                           ([128*128])
#   dice over masked objects -> scalar loss
#
# Strategy (v2):
#  * Host gathers the 593 params per object, packs active objects into groups
#    of 8 per image (G groups total).  x_rel/y_rel fold into b1_eff.
#  * Work unit = "wave" = 4 pixel-tiles of 512 px of one group.  Each group
#    has 32 tiles = 8 waves; wave j of every group goes to core j -> every
#    core gets exactly G waves (perfect balance).
#  * gemm1 runs as 16 concurrent 32x32 PE sub-array tiles: feat for the 4
#    pixel-tiles is STACKED in partition bands 32r (18 rows each, no
#    replication); MM (r,c) = tile r x object-pair c -> PSUM bank r holds
#    canonical h1 (obj o at partitions 16o) of tile r.
#  * gemm2 is block-diagonal per pair on the 4 diagonal sub-arrays (x,x),
#    reading canonical h1, writing canonical h2 -> bank r = tile r.
#  * evac1/evac2 are single [128,1024] relu+bias ops (fp32 PSUM -> fp16 SBUF),
#    alternating between DVE (tensor_scalar add+max) and ACT (activation Relu)
#    per half/wave so both engines stay saturated - they are the bottleneck.
#  * gemm3 accumulates 4 MMs per wave (one per tile, w3 block placed at
#    output columns 8r..8r+8) into a DENSE [32,512] pred quadrant: rows =
#    4 tiles x 8 objects, cols = 512 px of that row's tile.  4 waves fill a
#    [128,512] PSUM bank -> ONE sigmoid + 2 accumulating dice ops per 4 waves
#    (4x less pointwise work than per-tile layouts).
#  * Dice partials: sigmoid (ACT, bias b3, -50 on filler rows) then
#    scalar_tensor_tensor pred*tgt and pred*pred with accum_out (DVE, fp16
#    2x mode).  Host does the final tiny reduction + sum(tgt^2).
import math
import numpy as np
from contextlib import ExitStack

import concourse.bass as bass
import concourse.tile as tile
from concourse import mybir, bacc
from concourse.bass_utils import run_bass_kernel_spmd

C = 16
WT = 593
B, O, H, W = 4, 32, 128, 128
HW = H * W
N_CORES = 8
GRP = 8            # objects per group (4 pairs)
PXT = 512          # pixels per tile
TPW = 4            # tiles per wave
WPG = HW // (PXT * TPW)  # waves per group = 8 (== N_CORES)

F32 = mybir.dt.float32
F16 = mybir.dt.float16
ACTF = mybir.ActivationFunctionType
ALU = mybir.AluOpType

# per-group weight columns in the resident wtile (fp16):
#   0:128   w1pack: [32r+k, 32c+j] = w1^T of pair c (k<18), replicated per band r
#   128:160 w2pack: [32x+k, j]     = blockdiag(w2[2x]^T, w2[2x+1]^T)
#   160:288 w3pack: cols 32r+m     = lhsT3 variant r ([16o+ch, 8r+o] = w3[o,ch])
WCOL = 288


def host_pack(seg_feat, conv_weight, mask, ind, target):
    cw = conv_weight.reshape(B, WT, HW)
    weight = np.take_along_axis(cw, ind[:, None, :].astype(np.int64), axis=2)
    weight = np.ascontiguousarray(weight.transpose(0, 2, 1))  # [B, O, WT]
    s0 = (C + 2) * C
    w1 = weight[..., :s0].reshape(B, O, C, C + 2)
    b1 = weight[..., s0:s0 + C]
    w2 = weight[..., s0 + C:s0 + C + C * C].reshape(B, O, C, C)
    b2 = weight[..., s0 + C + C * C:s0 + 2 * C + C * C]
    w3 = weight[..., s0 + 2 * C + C * C:s0 + 3 * C + C * C]
    b3 = weight[..., -1]
    xo = (ind % W).astype(np.float32)
    yo = (ind // W).astype(np.float32)

    groups = []  # (b, [obj ids padded with -1])
    for b in range(B):
        objs = [o for o in range(O) if mask[b, o] == 1]
        for g0 in range(0, len(objs), GRP):
            grp = objs[g0:g0 + GRP]
            groups.append((b, grp + [-1] * (GRP - len(grp))))
    if not groups:
        groups.append((0, [-1] * GRP))
    G = len(groups)
    NB = (G + 3) // 4
    KLAST = G - 4 * (NB - 1)  # quadrants in last batch

    px = np.arange(HW, dtype=np.float32)
    xg = (px % W) / 128.0
    yg = np.floor(px / W) / 128.0
    tgt_flat = target.reshape(B, O, HW)

    # group-level packs (core independent)
    wall = np.zeros((G, 128, WCOL), np.float16)
    ball = np.zeros((G, 128, 3), np.float32)
    for g, (b, grp) in enumerate(groups):
        for oo, o in enumerate(grp):
            if o < 0:
                continue
            pr, sub = oo // 2, oo % 2  # pair index, slot in pair
            # w1pack: bands r, block col 32*pr, cols 16*sub..
            blk = w1[b, o].T.astype(np.float16)  # [18, 16]
            for r in range(4):
                wall[g, 32 * r:32 * r + 18, 32 * pr + 16 * sub:
                     32 * pr + 16 * sub + 16] = blk
            # w2pack at rows 32*pr (+16*sub), cols 128 + 16*sub
            wall[g, 32 * pr + 16 * sub:32 * pr + 16 * sub + 16,
                 128 + 16 * sub:128 + 16 * sub + 16] = \
                w2[b, o].T.astype(np.float16)
            b1e = (b1[b, o] - w1[b, o, :, 16] * (xo[b, o] / 128.0)
                   - w1[b, o, :, 17] * (yo[b, o] / 128.0))
            ball[g, 16 * oo:16 * oo + 16, 0] = b1e
            ball[g, 16 * oo:16 * oo + 16, 1] = b2[b, o]
            # rotated copy for the shifted-subarray h2b layout: pair p's
            # rows land at partition block (p+2)%4 for tiles 2,3
            ro = (oo + 4) % 8
            ball[g, 16 * ro:16 * ro + 16, 2] = b2[b, o]
    # w3pack: variant r lives at cols 160+32r..160+32r+32; within it the
    # nonzero column for (tile-slot r, object oo) is m = 8r+oo.
    for g, (b, grp) in enumerate(groups):
        for oo, o in enumerate(grp):
            if o < 0:
                continue
            for r in range(4):
                # for tiles r>=2, h2 lives in the rotated (p+2)%4 layout,
                # so the lhsT3 row block moves accordingly
                row0 = 16 * oo if r < 2 else 16 * ((oo + 4) % 8)
                wall[g, row0:row0 + 16,
                     160 + 32 * r + 8 * r + oo] = w3[b, o].astype(np.float16)

    # transpose-to-[128, G*...] layouts so weights/biases load in ONE DMA
    wall_t = np.ascontiguousarray(wall.transpose(1, 0, 2)).reshape(128, G * WCOL)
    ball_t = np.ascontiguousarray(ball.transpose(1, 0, 2)).reshape(128, G * 3)

    in_maps = []
    meta = {"groups": groups, "G": G, "NB": NB, "KLAST": KLAST}
    for ci in range(N_CORES):
        feat_pack = np.zeros((G, 128, PXT), np.float16)
        tgt_pack = np.zeros((NB, 128, PXT), np.float16)
        b3_pack = np.full((128, NB), -50.0, np.float32)
        for g, (b, grp) in enumerate(groups):
            for r in range(TPW):
                t = TPW * ci + r
                sl = slice(t * PXT, (t + 1) * PXT)
                feat_pack[g, 32 * r:32 * r + 16] = seg_feat[b].reshape(C, HW)[:, sl]
                feat_pack[g, 32 * r + 16] = xg[sl]
                feat_pack[g, 32 * r + 17] = yg[sl]
            nb, q = g // 4, g % 4
            for r in range(TPW):
                t = TPW * ci + r
                for oo, o in enumerate(grp):
                    if o < 0:
                        continue
                    row = 32 * q + 8 * r + oo
                    tgt_pack[nb, row] = tgt_flat[b, o, t * PXT:(t + 1) * PXT
                                                 ].astype(np.float16)
                    b3_pack[row, nb] = b3[b, o]
        in_maps.append({"feat": feat_pack, "wall": wall_t, "ball": ball_t,
                        "tgt": tgt_pack, "b3": b3_pack})
    return in_maps, meta


_PROGRAM_CACHE = {}


def build_program(G, NB, KLAST):
    key = (G, NB, KLAST)
    if key in _PROGRAM_CACHE:
        return _PROGRAM_CACHE[key]
    nc = bacc.Bacc("TRN2", target_bir_lowering=False, debug=False,
                   enable_asserts=False, num_devices=N_CORES)
    feat_t = nc.dram_tensor("feat", (G, 128, PXT), F16, kind="ExternalInput")
    wall_t = nc.dram_tensor("wall", (128, G * WCOL), F16, kind="ExternalInput")
    ball_t = nc.dram_tensor("ball", (128, G * 3), F32, kind="ExternalInput")
    tgt_t = nc.dram_tensor("tgt", (NB, 128, PXT), F16, kind="ExternalInput")
    b3_t = nc.dram_tensor("b3", (128, NB), F32, kind="ExternalInput")
    acc_t = nc.dram_tensor("acc", (128, 2 * NB), F32, kind="ExternalOutput")

    with tile.TileContext(nc) as tc, ExitStack() as ctx:
        wpool = ctx.enter_context(tc.tile_pool(name="wpool", bufs=1))
        fpool = ctx.enter_context(tc.tile_pool(name="fpool", bufs=4))
        h1pool = ctx.enter_context(tc.tile_pool(name="h1pool", bufs=2))
        h2pool = ctx.enter_context(tc.tile_pool(name="h2pool", bufs=5))
        spool = ctx.enter_context(tc.tile_pool(name="spool", bufs=2))
        apool = ctx.enter_context(tc.tile_pool(name="apool", bufs=1))
        ps1 = ctx.enter_context(tc.tile_pool(name="ps1", bufs=1, space="PSUM"))
        ps2 = ctx.enter_context(tc.tile_pool(name="ps2", bufs=1, space="PSUM"))

        acc_sb = apool.tile([128, 2 * NB], F32)
        inter_acc = acc_sb[:, 0:NB]
        psq_acc = acc_sb[:, NB:2 * NB]

        # wave-0 inputs first on the sync queue; per-group weight slices as
        # individual small DMAs so wave g's weights land long before wave g
        ft_tiles = {}
        ft_tiles[0] = fpool.tile([128, PXT], F16, tag="f", name="ft0")
        nc.sync.dma_start(out=ft_tiles[0], in_=feat_t.ap()[0])
        wts = []
        for g in range(G):
            wtg = wpool.tile([128, WCOL], F16, tag=f"w{g}", name="wtg")
            wts.append(wtg)
        bt = wpool.tile([128, 3 * G], F32)
        b3t = wpool.tile([128, NB], F32)
        nc.sync.dma_start(out=wts[0], in_=wall_t.ap()[:, 0:WCOL])
        nc.gpsimd.dma_start(out=bt, in_=ball_t.ap())
        nc.gpsimd.dma_start(out=b3t, in_=b3_t.ap())
        if G > 1:
            ft_tiles[1] = fpool.tile([128, PXT], F16, tag="f", name="ft1")
            nc.gpsimd.dma_start(out=ft_tiles[1], in_=feat_t.ap()[1])
        for g in range(1, G):
            nc.sync.dma_start(out=wts[g],
                              in_=wall_t.ap()[:, WCOL * g:WCOL * (g + 1)])

        def wslice(g, lo, hi):
            return wts[g][:, lo:hi]

        # dice-batch inputs prefetched on the gpsimd queue
        tg_tiles = []
        for nb in range(NB):
            tg = spool.tile([128, PXT], F16, tag="t", bufs=NB, name="tg")
            nc.gpsimd.dma_start(out=tg, in_=tgt_t.ap()[nb])
            tg_tiles.append(tg)

        # sigmoid table set (covers relu/square) loads on ACT during the
        # initial DMA wait
        scr = apool.tile([128, 512], F16)
        nc.vector.memset(scr[0:1, 0:1], 0.125)
        scr1 = apool.tile([128, 1], F32)
        nc.scalar.activation(scr1[0:1, :], scr[0:1, 0:1], ACTF.Sigmoid,
                             bias=0.0, scale=1.0)

        def evac_half(engine, dst, src, bias_ap):
            if engine == 0:
                nc.vector.tensor_scalar(out=dst, in0=src, scalar1=bias_ap,
                                        scalar2=0.0, op0=ALU.add, op1=ALU.max)
            else:
                nc.scalar.activation(dst, src, ACTF.Relu, bias=bias_ap,
                                     scale=1.0)

        h1_of = {}
        h2_tiles = [None] * 4
        p1_of = {}

        # Software-pipelined: iteration w runs gemm1/evac1 of wave w and
        # gemm2/evac2 (+dice batch) of wave w-1, so each PE burst
        # (g1a,g1b,g2a,g2b) hides under the previous engine phase.
        for w in range(G + 1):
            if w + 2 < G:
                ftn = fpool.tile([128, PXT], F16, tag="f", name="ftn")
                if w % 2 == 0:
                    nc.sync.dma_start(out=ftn, in_=feat_t.ap()[w + 2])
                else:
                    nc.gpsimd.dma_start(out=ftn, in_=feat_t.ap()[w + 2])
                ft_tiles[w + 2] = ftn

            if w < G:
                ft = ft_tiles.pop(w)
                p1a = ps1.tile([128, 1024], F32, tag="g1a", name="p1a")
                p1b = ps1.tile([128, 1024], F32, tag="g1b", name="p1b")
                for r in range(TPW):
                    p1s = p1a if r < 2 else p1b
                    for c2 in range(4):
                        nc.tensor.matmul(
                            p1s[32 * c2:32 * c2 + 32,
                                512 * (r % 2):512 * (r % 2) + 512],
                            wslice(w, 0, 128)[32 * r:32 * r + 18,
                                              32 * c2:32 * c2 + 32],
                            ft[32 * r:32 * r + 18, :],
                            start=True, stop=True,
                            tile_position=(32 * r, 32 * c2))
                p1_of[w] = (p1a, p1b)

            if w > 0:
                v = w - 1
                h1a, h1b = h1_of[v]
                p2a = ps2.tile([128, 1024], F32, tag="g2a", name="p2a")
                p2b = ps2.tile([128, 1024], F32, tag="g2b", name="p2b")
                for r in range(TPW):
                    h1s = h1a if r < 2 else h1b
                    p2s = p2a if r < 2 else p2b
                    cc = 512 * (r % 2)
                    for x in range(4):
                        # tiles 2,3 use the shifted column-group (x+2)%4 so
                        # the 8 g2a MMs and 8 g2b MMs occupy disjoint PE
                        # sub-arrays and run concurrently (halves the
                        # diagonal-chain span)
                        y = x if r < 2 else (x + 2) % 4
                        nc.tensor.matmul(
                            p2s[32 * y:32 * y + 32, cc:cc + 512],
                            wslice(v, 128, 160)[32 * x:32 * x + 32, :],
                            h1s[32 * x:32 * x + 32, cc:cc + 512],
                            start=True, stop=True,
                            tile_position=(32 * x, 32 * y))

            if w < G:
                b1ap = bt[:, 3 * w:3 * w + 1]
                h1a = h1pool.tile([128, 1024], F16, tag="h1a", name="h1a")
                h1b = h1pool.tile([128, 1024], F16, tag="h1b", name="h1b")
                p1a, p1b = p1_of.pop(w)
                drop_dve = (w % 8 == 6)
                evac_half(1 if drop_dve else 0, h1a, p1a, b1ap)
                evac_half(1, h1b, p1b, b1ap)
                h1_of[w] = (h1a, h1b)

            if w > 0:
                v = w - 1
                b2ap = bt[:, 3 * v + 1:3 * v + 2]
                b2rap = bt[:, 3 * v + 2:3 * v + 3]
                h2a = h2pool.tile([128, 1024], F16, tag="h2a", name="h2a")
                h2b = h2pool.tile([128, 1024], F16, tag="h2b", name="h2b")
                evac_half(0, h2a, p2a, b2ap)
                evac_half(1, h2b, p2b, b2rap)
                h2_tiles[v % 4] = (h2a, h2b, v)

                if v % 4 == 3 or v == G - 1:
                    nb = v // 4
                    k = v % 4 + 1
                    pred = ps2.tile([128, 512], F32, tag="g2a", name="pred")
                    for q in range(k):
                        h2qa, h2qb, gq = h2_tiles[q]
                        for r in range(TPW):
                            h2s = h2qa if r < 2 else h2qb
                            cc = 512 * (r % 2)
                            nc.tensor.matmul(
                                pred[32 * q:32 * q + 32, :],
                                wslice(gq, 160 + 32 * r, 192 + 32 * r),
                                h2s[:, cc:cc + 512],
                                start=(r == 0), stop=(r == TPW - 1),
                                tile_position=(0, 32 * q))
                    tg = tg_tiles[nb]
                    predsb = spool.tile([128, PXT], F16, tag="p", name="psb")
                    pp = 32 * k
                    nc.scalar.activation(predsb[0:pp, :], pred[0:pp, :],
                                         ACTF.Sigmoid,
                                         bias=b3t[0:pp, nb:nb + 1], scale=1.0)
                    sc1 = spool.tile([128, PXT], F16, tag="s1", name="sc1")
                    nc.vector.scalar_tensor_tensor(
                        out=sc1[0:pp, :], in0=predsb[0:pp, :], scalar=0.0,
                        in1=tg[0:pp, :], op0=ALU.add, op1=ALU.mult,
                        accum_out=inter_acc[0:pp, nb:nb + 1])
                    sc2 = spool.tile([128, PXT], F16, tag="s2", name="sc2")
                    nc.scalar.activation(sc2[0:pp, :], predsb[0:pp, :],
                                         ACTF.Square,
                                         accum_out=psq_acc[0:pp, nb:nb + 1])

        nc.sync.dma_start(out=acc_t.ap(), in_=acc_sb)

    nc.compile()
    _PROGRAM_CACHE[key] = nc
    return nc


def _run(inputs, trace=False):
    seg_feat = np.asarray(inputs["seg_feat"], np.float32)
    conv_weight = np.asarray(inputs["conv_weight"], np.float32)
    mask = np.asarray(inputs["mask"])
    ind = np.asarray(inputs["ind"])
    target = np.asarray(inputs["target"], np.float32)

    in_maps, meta = host_pack(seg_feat, conv_weight, mask, ind, target)
    G, NB, KLAST = meta["G"], meta["NB"], meta["KLAST"]
    groups = meta["groups"]
    nc = build_program(G, NB, KLAST)
    res = run_bass_kernel_spmd(nc, in_maps, core_ids=list(range(N_CORES)),
                               trace=trace)

    inter = np.zeros(B, np.float64)
    predsq = np.zeros(B, np.float64)
    for ci in range(N_CORES):
        acc = res.results[ci]["acc"]
        for g, (b, grp) in enumerate(groups):
            if all(o < 0 for o in grp):
                continue
            nb, q = g // 4, g % 4
            inter[b] += acc[32 * q:32 * q + 32, nb].sum(dtype=np.float64)
            predsq[b] += acc[32 * q:32 * q + 32, NB + nb].sum(dtype=np.float64)
    tgtsq = ((target.reshape(B, O, HW).astype(np.float64) ** 2)
             * mask[:, :, None]).sum(axis=(1, 2))
    loss = 1.0 - (2.0 * inter + 1.0) / (predsq + tgtsq + 1.0)
    return np.float32(loss.mean()), res


def kernel(**inputs):
    loss, _ = _run(inputs, trace=False)
    return np.array(loss, dtype=np.float32)


# revision 45
# speedup vs baseline: 1.0356x; 1.0248x over previous
# Fused dynamic-conv (CondInst-style) + dice loss kernel for 8x TRN2 NeuronCores.
#
# Reference computation (per batch image b, object o):
#   weight[b,o,:] = conv_weight[b, :, ind[b,o]]           (gather, 593 params)
#   feat = concat(seg_feat[b], x_rel(o), y_rel(o))        ([18, 128*128])
#   h1 = relu(w1 @ feat + b1); h2 = relu(w2 @ h1 + b2)    (16-ch dynamic 1x1 convs)
#   out = sigmoid(w3 . h2 + b3)                           ([128*128])
#   dice over masked objects -> scalar loss
#
# Strategy (v2):
#  * Host gathers the 593 params per object, packs active objects into groups
#    of 8 per image (G groups total).  x_rel/y_rel fold into b1_eff.
#  * Work unit = "wave" = 4 pixel-tiles of 512 px of one group.  Each group
#    has 32 tiles = 8 waves; wave j of every group goes to core j -> every
#    core gets exactly G waves (perfect balance).
#  * gemm1 runs as 16 concurrent 32x32 PE sub-array tiles: feat for the 4
#    pixel-tiles is STACKED in partition bands 32r (18 rows each, no
#    replication); MM (r,c) = tile r x object-pair c -> PSUM bank r holds
#    canonical h1 (obj o at partitions 16o) of tile r.
#  * gemm2 is block-diagonal per pair on the 4 diagonal sub-arrays (x,x),
#    reading canonical h1, writing canonical h2 -> bank r = tile r.
#  * evac1/evac2 are single [128,1024] relu+bias ops (fp32 PSUM -> fp16 SBUF),
#    alternating between DVE (tensor_scalar add+max) and ACT (activation Relu)
#    per half/wave so both engines stay saturated - they are the bottleneck.
#  * gemm3 accumulates 4 MMs per wave (one per tile, w3 block placed at
#    output columns 8r..8r+8) into a DENSE [32,512] pred quadrant: rows =
#    4 tiles x 8 objects, cols = 512 px of that row's tile.  4 waves fill a
#    [128,512] PSUM bank -> ONE sigmoid + 2 accumulating dice ops per 4 waves
#    (4x less pointwise work than per-tile layouts).
#  * Dice partials: sigmoid (ACT, bias b3, -50 on filler rows) then
#    scalar_tensor_tensor pred*tgt and pred*pred with accum_out (DVE, fp16
#    2x mode).  Host does the final tiny reduction + sum(tgt^2).
import math
import numpy as np
from contextlib import ExitStack

import concourse.bass as bass
import concourse.tile as tile
from concourse import mybir, bacc
from concourse.bass_utils import run_bass_kernel_spmd

C = 16
WT = 593
B, O, H, W = 4, 32, 128, 128
HW = H * W
N_CORES = 8
GRP = 8            # objects per group (4 pairs)
PXT = 512          # pixels per tile
TPW = 4            # tiles per wave
WPG = HW // (PXT * TPW)  # waves per group = 8 (== N_CORES)

F32 = mybir.dt.float32
F16 = mybir.dt.float16
ACTF = mybir.ActivationFunctionType
ALU = mybir.AluOpType

# per-group weight columns in the resident wtile (fp16):
#   0:128   w1pack: [32r+k, 32c+j] = w1^T of pair c (k<18), replicated per band r
#   128:160 w2pack: [32x+k, j]     = blockdiag(w2[2x]^T, w2[2x+1]^T)
#   160:288 w3pack: cols 32r+m     = lhsT3 variant r ([16o+ch, 8r+o] = w3[o,ch])
WCOL = 288


def host_pack(seg_feat, conv_weight, mask, ind, target):
    cw = conv_weight.reshape(B, WT, HW)
    weight = np.take_along_axis(cw, ind[:, None, :].astype(np.int64), axis=2)
    weight = np.ascontiguousarray(weight.transpose(0, 2, 1))  # [B, O, WT]
    s0 = (C + 2) * C
    w1 = weight[..., :s0].reshape(B, O, C, C + 2)
    b1 = weight[..., s0:s0 + C]
    w2 = weight[..., s0 + C:s0 + C + C * C].reshape(B, O, C, C)
    b2 = weight[..., s0 + C + C * C:s0 + 2 * C + C * C]
    w3 = weight[..., s0 + 2 * C + C * C:s0 + 3 * C + C * C]
    b3 = weight[..., -1]
    xo = (ind % W).astype(np.float32)
    yo = (ind // W).astype(np.float32)

    groups = []  # (b, [obj ids padded with -1])
    for b in range(B):
        objs = [o for o in range(O) if mask[b, o] == 1]
        for g0 in range(0, len(objs), GRP):
            grp = objs[g0:g0 + GRP]
            groups.append((b, grp + [-1] * (GRP - len(grp))))
    if not groups:
        groups.append((0, [-1] * GRP))
    G = len(groups)
    NB = (G + 3) // 4
    KLAST = G - 4 * (NB - 1)  # quadrants in last batch

    px = np.arange(HW, dtype=np.float32)
    xg = (px % W) / 128.0
    yg = np.floor(px / W) / 128.0
    tgt_flat = target.reshape(B, O, HW)

    # group-level packs (core independent)
    wall = np.zeros((G, 128, WCOL), np.float16)
    ball = np.zeros((G, 128, 2), np.float32)
    for g, (b, grp) in enumerate(groups):
        for oo, o in enumerate(grp):
            if o < 0:
                continue
            pr, sub = oo // 2, oo % 2  # pair index, slot in pair
            # w1pack: bands r, block col 32*pr, cols 16*sub..
            blk = w1[b, o].T.astype(np.float16)  # [18, 16]
            for r in range(4):
                wall[g, 32 * r:32 * r + 18, 32 * pr + 16 * sub:
                     32 * pr + 16 * sub + 16] = blk
            # w2pack at rows 32*pr (+16*sub), cols 128 + 16*sub
            wall[g, 32 * pr + 16 * sub:32 * pr + 16 * sub + 16,
                 128 + 16 * sub:128 + 16 * sub + 16] = \
                w2[b, o].T.astype(np.float16)
            b1e = (b1[b, o] - w1[b, o, :, 16] * (xo[b, o] / 128.0)
                   - w1[b, o, :, 17] * (yo[b, o] / 128.0))
            ball[g, 16 * oo:16 * oo + 16, 0] = b1e
            ball[g, 16 * oo:16 * oo + 16, 1] = b2[b, o]
    # w3pack: variant r lives at cols 160+32r..160+32r+32; within it the
    # nonzero column for (tile-slot r, object oo) is m = 8r+oo.
    for g, (b, grp) in enumerate(groups):
        for oo, o in enumerate(grp):
            if o < 0:
                continue
            for r in range(4):
                wall[g, 16 * oo:16 * oo + 16,
                     160 + 32 * r + 8 * r + oo] = w3[b, o].astype(np.float16)

    # transpose-to-[128, G*...] layouts so weights/biases load in ONE DMA
    wall_t = np.ascontiguousarray(wall.transpose(1, 0, 2)).reshape(128, G * WCOL)
    ball_t = np.ascontiguousarray(ball.transpose(1, 0, 2)).reshape(128, G * 2)

    in_maps = []
    meta = {"groups": groups, "G": G, "NB": NB, "KLAST": KLAST}
    for ci in range(N_CORES):
        feat_pack = np.zeros((G, 128, PXT), np.float16)
        tgt_pack = np.zeros((NB, 128, PXT), np.float16)
        b3_pack = np.full((128, NB), -50.0, np.float32)
        for g, (b, grp) in enumerate(groups):
            for r in range(TPW):
                t = TPW * ci + r
                sl = slice(t * PXT, (t + 1) * PXT)
                feat_pack[g, 32 * r:32 * r + 16] = seg_feat[b].reshape(C, HW)[:, sl]
                feat_pack[g, 32 * r + 16] = xg[sl]
                feat_pack[g, 32 * r + 17] = yg[sl]
            nb, q = g // 4, g % 4
            for r in range(TPW):
                t = TPW * ci + r
                for oo, o in enumerate(grp):
                    if o < 0:
                        continue
                    row = 32 * q + 8 * r + oo
                    tgt_pack[nb, row] = tgt_flat[b, o, t * PXT:(t + 1) * PXT
                                                 ].astype(np.float16)
                    b3_pack[row, nb] = b3[b, o]
        in_maps.append({"feat": feat_pack, "wall": wall_t, "ball": ball_t,
                        "tgt": tgt_pack, "b3": b3_pack})
    return in_maps, meta


_PROGRAM_CACHE = {}


def build_program(G, NB, KLAST):
    key = (G, NB, KLAST)
    if key in _PROGRAM_CACHE:
        return _PROGRAM_CACHE[key]
    nc = bacc.Bacc("TRN2", target_bir_lowering=False, debug=False,
                   enable_asserts=False, num_devices=N_CORES)
    feat_t = nc.dram_tensor("feat", (G, 128, PXT), F16, kind="ExternalInput")
    wall_t = nc.dram_tensor("wall", (128, G * WCOL), F16, kind="ExternalInput")
    ball_t = nc.dram_tensor("ball", (128, G * 2), F32, kind="ExternalInput")
    tgt_t = nc.dram_tensor("tgt", (NB, 128, PXT), F16, kind="ExternalInput")
    b3_t = nc.dram_tensor("b3", (128, NB), F32, kind="ExternalInput")
    acc_t = nc.dram_tensor("acc", (128, 2 * NB), F32, kind="ExternalOutput")

    with tile.TileContext(nc) as tc, ExitStack() as ctx:
        wpool = ctx.enter_context(tc.tile_pool(name="wpool", bufs=1))
        fpool = ctx.enter_context(tc.tile_pool(name="fpool", bufs=4))
        h1pool = ctx.enter_context(tc.tile_pool(name="h1pool", bufs=2))
        h2pool = ctx.enter_context(tc.tile_pool(name="h2pool", bufs=5))
        spool = ctx.enter_context(tc.tile_pool(name="spool", bufs=2))
        apool = ctx.enter_context(tc.tile_pool(name="apool", bufs=1))
        ps1 = ctx.enter_context(tc.tile_pool(name="ps1", bufs=1, space="PSUM"))
        ps2 = ctx.enter_context(tc.tile_pool(name="ps2", bufs=1, space="PSUM"))

        acc_sb = apool.tile([128, 2 * NB], F32)
        inter_acc = acc_sb[:, 0:NB]
        psq_acc = acc_sb[:, NB:2 * NB]

        # wave-0 inputs first on the sync queue; per-group weight slices as
        # individual small DMAs so wave g's weights land long before wave g
        ft_tiles = {}
        ft_tiles[0] = fpool.tile([128, PXT], F16, tag="f", name="ft0")
        nc.sync.dma_start(out=ft_tiles[0], in_=feat_t.ap()[0])
        wts = []
        for g in range(G):
            wtg = wpool.tile([128, WCOL], F16, tag=f"w{g}", name="wtg")
            wts.append(wtg)
        bt = wpool.tile([128, 2 * G], F32)
        b3t = wpool.tile([128, NB], F32)
        nc.sync.dma_start(out=wts[0], in_=wall_t.ap()[:, 0:WCOL])
        nc.gpsimd.dma_start(out=bt, in_=ball_t.ap())
        nc.gpsimd.dma_start(out=b3t, in_=b3_t.ap())
        if G > 1:
            ft_tiles[1] = fpool.tile([128, PXT], F16, tag="f", name="ft1")
            nc.gpsimd.dma_start(out=ft_tiles[1], in_=feat_t.ap()[1])
        for g in range(1, G):
            nc.sync.dma_start(out=wts[g],
                              in_=wall_t.ap()[:, WCOL * g:WCOL * (g + 1)])

        def wslice(g, lo, hi):
            return wts[g][:, lo:hi]

        # dice-batch inputs prefetched on the gpsimd queue
        tg_tiles = []
        for nb in range(NB):
            tg = spool.tile([128, PXT], F16, tag="t", bufs=NB, name="tg")
            nc.gpsimd.dma_start(out=tg, in_=tgt_t.ap()[nb])
            tg_tiles.append(tg)

        # sigmoid table set (covers relu/square) loads on ACT during the
        # initial DMA wait
        scr = apool.tile([128, 512], F16)
        nc.vector.memset(scr[0:1, 0:1], 0.125)
        scr1 = apool.tile([128, 1], F32)
        nc.scalar.activation(scr1[0:1, :], scr[0:1, 0:1], ACTF.Sigmoid,
                             bias=0.0, scale=1.0)

        def evac_half(engine, dst, src, bias_ap):
            if engine == 0:
                nc.vector.tensor_scalar(out=dst, in0=src, scalar1=bias_ap,
                                        scalar2=0.0, op0=ALU.add, op1=ALU.max)
            else:
                nc.scalar.activation(dst, src, ACTF.Relu, bias=bias_ap,
                                     scale=1.0)

        h1_of = {}
        h2_tiles = [None] * 4
        p1_of = {}

        # Software-pipelined: iteration w runs gemm1/evac1 of wave w and
        # gemm2/evac2 (+dice batch) of wave w-1, so each PE burst
        # (g1a,g1b,g2a,g2b) hides under the previous engine phase.
        for w in range(G + 1):
            if w + 2 < G:
                ftn = fpool.tile([128, PXT], F16, tag="f", name="ftn")
                if w % 2 == 0:
                    nc.sync.dma_start(out=ftn, in_=feat_t.ap()[w + 2])
                else:
                    nc.gpsimd.dma_start(out=ftn, in_=feat_t.ap()[w + 2])
                ft_tiles[w + 2] = ftn

            if w < G:
                ft = ft_tiles.pop(w)
                p1a = ps1.tile([128, 1024], F32, tag="g1a", name="p1a")
                p1b = ps1.tile([128, 1024], F32, tag="g1b", name="p1b")
                for r in range(TPW):
                    p1s = p1a if r < 2 else p1b
                    for c2 in range(4):
                        nc.tensor.matmul(
                            p1s[32 * c2:32 * c2 + 32,
                                512 * (r % 2):512 * (r % 2) + 512],
                            wslice(w, 0, 128)[32 * r:32 * r + 18,
                                              32 * c2:32 * c2 + 32],
                            ft[32 * r:32 * r + 18, :],
                            start=True, stop=True,
                            tile_position=(32 * r, 32 * c2))
                p1_of[w] = (p1a, p1b)

            if w > 0:
                v = w - 1
                h1a, h1b = h1_of[v]
                p2a = ps2.tile([128, 1024], F32, tag="g2a", name="p2a")
                p2b = ps2.tile([128, 1024], F32, tag="g2b", name="p2b")
                for r in range(TPW):
                    h1s = h1a if r < 2 else h1b
                    p2s = p2a if r < 2 else p2b
                    cc = 512 * (r % 2)
                    for x in range(4):
                        nc.tensor.matmul(
                            p2s[32 * x:32 * x + 32, cc:cc + 512],
                            wslice(v, 128, 160)[32 * x:32 * x + 32, :],
                            h1s[32 * x:32 * x + 32, cc:cc + 512],
                            start=True, stop=True,
                            tile_position=(32 * x, 32 * x))

            if w < G:
                b1ap = bt[:, 2 * w:2 * w + 1]
                h1a = h1pool.tile([128, 1024], F16, tag="h1a", name="h1a")
                h1b = h1pool.tile([128, 1024], F16, tag="h1b", name="h1b")
                p1a, p1b = p1_of.pop(w)
                drop_dve = (w % 8 == 6)
                evac_half(1 if drop_dve else 0, h1a, p1a, b1ap)
                evac_half(1, h1b, p1b, b1ap)
                h1_of[w] = (h1a, h1b)

            if w > 0:
                v = w - 1
                b2ap = bt[:, 2 * v + 1:2 * v + 2]
                h2a = h2pool.tile([128, 1024], F16, tag="h2a", name="h2a")
                h2b = h2pool.tile([128, 1024], F16, tag="h2b", name="h2b")
                evac_half(0, h2a, p2a, b2ap)
                evac_half(1, h2b, p2b, b2ap)
                h2_tiles[v % 4] = (h2a, h2b, v)

                if v % 4 == 3 or v == G - 1:
                    nb = v // 4
                    k = v % 4 + 1
                    if v == G - 1:
                        # last batch: gemm1 is done, so borrow a ps1 slot -
                        # the pred tile is then free right after e1a(G-1) and
                        # quadrants 0..k-2 overlap the final waves instead of
                        # trailing evac2(G-1)
                        pred = ps1.tile([128, 512], F32, tag="g1a", name="pred")
                    else:
                        pred = ps2.tile([128, 512], F32, tag="g2a", name="pred")
                    for q in range(k):
                        h2qa, h2qb, gq = h2_tiles[q]
                        for r in range(TPW):
                            h2s = h2qa if r < 2 else h2qb
                            cc = 512 * (r % 2)
                            nc.tensor.matmul(
                                pred[32 * q:32 * q + 32, :],
                                wslice(gq, 160 + 32 * r, 192 + 32 * r),
                                h2s[:, cc:cc + 512],
                                start=(r == 0), stop=(r == TPW - 1),
                                tile_position=(0, 32 * q))
                    tg = tg_tiles[nb]
                    predsb = spool.tile([128, PXT], F16, tag="p", name="psb")
                    pp = 32 * k
                    nc.scalar.activation(predsb[0:pp, :], pred[0:pp, :],
                                         ACTF.Sigmoid,
                                         bias=b3t[0:pp, nb:nb + 1], scale=1.0)
                    sc1 = spool.tile([128, PXT], F16, tag="s1", name="sc1")
                    nc.vector.scalar_tensor_tensor(
                        out=sc1[0:pp, :], in0=predsb[0:pp, :], scalar=0.0,
                        in1=tg[0:pp, :], op0=ALU.add, op1=ALU.mult,
                        accum_out=inter_acc[0:pp, nb:nb + 1])
                    sc2 = spool.tile([128, PXT], F16, tag="s2", name="sc2")
                    nc.scalar.activation(sc2[0:pp, :], predsb[0:pp, :],
                                         ACTF.Square,
                                         accum_out=psq_acc[0:pp, nb:nb + 1])

        nc.sync.dma_start(out=acc_t.ap(), in_=acc_sb)

    nc.compile()
    _PROGRAM_CACHE[key] = nc
    return nc


def _run(inputs, trace=False):
    seg_feat = np.asarray(inputs["seg_feat"], np.float32)
    conv_weight = np.asarray(inputs["conv_weight"], np.float32)
    mask = np.asarray(inputs["mask"])
    ind = np.asarray(inputs["ind"])
    target = np.asarray(inputs["target"], np.float32)

    in_maps, meta = host_pack(seg_feat, conv_weight, mask, ind, target)
    G, NB, KLAST = meta["G"], meta["NB"], meta["KLAST"]
    groups = meta["groups"]
    nc = build_program(G, NB, KLAST)
    res = run_bass_kernel_spmd(nc, in_maps, core_ids=list(range(N_CORES)),
                               trace=trace)

    inter = np.zeros(B, np.float64)
    predsq = np.zeros(B, np.float64)
    for ci in range(N_CORES):
        acc = res.results[ci]["acc"]
        for g, (b, grp) in enumerate(groups):
            if all(o < 0 for o in grp):
                continue
            nb, q = g // 4, g % 4
            inter[b] += acc[32 * q:32 * q + 32, nb].sum(dtype=np.float64)
            predsq[b] += acc[32 * q:32 * q + 32, NB + nb].sum(dtype=np.float64)
    tgtsq = ((target.reshape(B, O, HW).astype(np.float64) ** 2)
             * mask[:, :, None]).sum(axis=(1, 2))
    loss = 1.0 - (2.0 * inter + 1.0) / (predsq + tgtsq + 1.0)
    return np.float32(loss.mean()), res


def kernel(**inputs):
    loss, _ = _run(inputs, trace=False)
    return np.array(loss, dtype=np.float32)
